# revision 16
# baseline (speedup 1.0000x reference)
"""Trainium2 Bass kernel for nn_MultiHeadAttention_49770081026139.

Multi-head attention with an edge tensor:
    qh = (q @ Wq + bq) * d^-0.5 ; kh = k @ Wk + bk ; vh = v @ Wv + bv
    eh = e @ We + be                      (b, i, j, H) -> heads (b, h, i, j, d)
    qk_e[b,h,i,j,d] = qh[b,h,i,d] * kh[b,h,j,d] * eh[b,h,i,j,d]
    w  = qk_e.sum(d) + attn_bias ; a = softmax_j(w)
    x  = (a @ vh) combined-heads @ Wo + bo            -> (b, n, H)
    w_out = qk_e rearranged (b,i,j,(h d)) @ Weo + beo -> (b, n, n, H)

Sharding: 8 cores = 4 batches x 2 halves of the i axis. Fully data
parallel (softmax is over j, kept whole per core) - no collectives.

On-device layout is "feature major": channels live on SBUF partitions
(4 chunks of 128) so every linear layer is a PE matmul with the weight
chunk [cin,cout] stationary, and the per-i broadcast of qh becomes a
per-partition tensor_scalar. e tiles are cast to bf16 during the DMA
(SWDGE cast) and transposed on the PE. Big matmuls run in bf16
(1 cyc/row) with fp32 PSUM accumulation. w_out is stored bf16 and
upcast on the host; bo/beo (additive, output-side) are applied on the
host only if nonzero.
"""

import numpy as np
import ml_dtypes

BF = ml_dtypes.bfloat16

B, NTOK, HID = 4, 256, 512
NHEAD, DHEAD = 16, 32
ILOC = 128          # i rows per core
NJ = 256            # full j per core
NCH = 4             # channel chunks (512 / 128)
NCORES = 8

_last_results = None  # stash of BassKernelResults for test harness introspection


def _split_multi_waits(mod):
    """This container's walrus accepts at most one sync-wait per instruction;
    Tile's scheduler embeds several. Hoist extras into standalone
    EventSemaphore waits on the same engine immediately before the
    instruction (same-engine program order makes this equivalent)."""
    for fn in mod["functions"]:
        for blk in fn["blocks"]:
            out = []
            for inst in blk["instructions"]:
                si = inst.get("sync_info")
                waits = (si or {}).get("on_wait") or []
                limit = 0 if inst.get("opcode") == "DMACopy" else 1
                if len(waits) > limit:
                    keep = waits[len(waits) - limit:]
                    hoist = waits[:len(waits) - limit]
                    for k, w in enumerate(hoist):
                        out.append({
                            "debug": inst.get("debug", 0),
                            "engine": inst["engine"],
                            "ins": [], "outs": [],
                            "name": f"{inst['name']}.w{k}",
                            "opcode": "EventSemaphore",
                            "sync_info": {"on_update": [], "on_wait": [w]},
                        })
                    si["on_wait"] = keep
                out.append(inst)
            blk["instructions"] = out
    return mod


def _patch_json_serialization(nc):
    import orjson

    orig = nc.to_json_bytes

    def patched():
        return orjson.dumps(_split_multi_waits(orjson.loads(orig())))

    nc.to_json_bytes = patched
    return nc


def build_nc(n_i=ILOC):
    """Build the per-core Bass program (SPMD: same program, different data)."""
    from contextlib import ExitStack

    import concourse.bass as bass
    import concourse.mybir as mybir
    import concourse.tile as tile

    f32 = mybir.dt.float32
    bf = mybir.dt.bfloat16
    AX = mybir.AxisListType
    ALU = mybir.AluOpType
    ACTF = mybir.ActivationFunctionType

    assert n_i % 4 == 0
    nblk = n_i // 4

    nc = bass.Bass()

    e_d = nc.dram_tensor("e_loc", [n_i, NJ, HID], f32, kind="ExternalInput")
    q_d = nc.dram_tensor("q_loc", [ILOC, HID], f32, kind="ExternalInput")
    k_d = nc.dram_tensor("k_loc", [NJ, HID], f32, kind="ExternalInput")
    v_d = nc.dram_tensor("v_loc", [NJ, HID], f32, kind="ExternalInput")
    bias_d = nc.dram_tensor("bias_loc", [nblk, 128, NJ], f32, kind="ExternalInput")
    w_names = ["wq", "wk", "wv", "wo", "we", "weo"]
    w_d = {n: nc.dram_tensor(n, [128, NCH, HID], bf, kind="ExternalInput") for n in w_names}
    ident_d = nc.dram_tensor("ident", [128, 128], bf, kind="ExternalInput")
    blk_d = nc.dram_tensor("blk", [128, NCH * NHEAD], bf, kind="ExternalInput")
    brows_d = nc.dram_tensor("brows", [1, 4, HID], bf, kind="ExternalInput")
    becol_d = nc.dram_tensor("becol", [128, NCH], f32, kind="ExternalInput")

    wout_d = nc.dram_tensor("wout_loc", [n_i, NJ, HID], bf, kind="ExternalOutput")
    x_d = nc.dram_tensor("x_loc", [ILOC, HID], f32, kind="ExternalOutput")

    with tile.TileContext(nc) as tc, ExitStack() as ctx:
        consts = ctx.enter_context(tc.tile_pool(name="consts", bufs=1))
        prep = ctx.enter_context(tc.tile_pool(name="prep", bufs=1))
        p_e = ctx.enter_context(tc.tile_pool(name="p_e", bufs=2))
        p_ef = ctx.enter_context(tc.tile_pool(name="p_ef", bufs=3))
        p_eT = ctx.enter_context(tc.tile_pool(name="p_eT", bufs=2))
        p_ehb = ctx.enter_context(tc.tile_pool(name="p_ehb", bufs=2))
        p_qk = ctx.enter_context(tc.tile_pool(name="p_qk", bufs=2))
        p_wout = ctx.enter_context(tc.tile_pool(name="p_wout", bufs=2))
        p_bias = ctx.enter_context(tc.tile_pool(name="p_bias", bufs=2))
        p_sm = ctx.enter_context(tc.tile_pool(name="p_sm", bufs=2))
        p_small = ctx.enter_context(tc.tile_pool(name="p_small", bufs=4))
        psA = ctx.enter_context(tc.tile_pool(name="psA", bufs=2, space="PSUM"))
        psW = ctx.enter_context(tc.tile_pool(name="psW", bufs=1, space="PSUM"))
        psB = ctx.enter_context(tc.tile_pool(name="psB", bufs=1, space="PSUM"))
        psX = ctx.enter_context(tc.tile_pool(name="psX", bufs=1, space="PSUM"))

        # ---- constants -------------------------------------------------
        ident_sb = consts.tile([128, 128], bf)
        nc.sync.dma_start(ident_sb, ident_d[:, :])
        blk_sb = consts.tile([128, NCH * NHEAD], bf)
        nc.sync.dma_start(blk_sb, blk_d[:, :])
        brows_sb = consts.tile([1, 4, HID], bf)
        nc.sync.dma_start(brows_sb, brows_d[:, :, :])
        becol_sb = consts.tile([128, NCH], f32)
        nc.sync.dma_start(becol_sb, becol_d[:, :])
        ones_sb = consts.tile([1, NJ], bf)
        nc.vector.memset(ones_sb, 1.0)
        w_sbs = {}
        for n in w_names:
            w_sbs[n] = consts.tile([128, NCH, HID], bf, name=n, tag=n)
            nc.sync.dma_start(w_sbs[n], w_d[n][:, :, :])

        # ---- prep: project q, k, v ------------------------------------
        # q -> qhT_sb [cout%128, (chunk, i)] fp32 (per-partition scalars)
        q_sb = prep.tile([128, HID], f32)
        nc.sync.dma_start(q_sb, q_d[:, :])
        q_bf = prep.tile([128, HID], bf)
        nc.scalar.copy(q_bf, q_sb)
        ps_t = psB.tile([128, NCH, 128], bf, tag="ehT")
        for r in range(NCH):
            nc.tensor.transpose(ps_t[:, r, :], q_bf[:, r * 128:(r + 1) * 128], ident_sb)
        qT_bf = prep.tile([128, NCH, 128], bf)
        nc.vector.tensor_copy(qT_bf, ps_t)
        ps_qh = psB.tile([128, NCH, 128], f32, tag="ehT")
        for ro in range(NCH):
            for ri in range(NCH):
                nc.tensor.matmul(ps_qh[:, ro, :], w_sbs["wq"][:, ri, ro * 128:(ro + 1) * 128],
                                 qT_bf[:, ri, :], start=(ri == 0), stop=False)
            nc.tensor.matmul(ps_qh[:, ro, :], brows_sb[0:1, 0, ro * 128:(ro + 1) * 128],
                             ones_sb[0:1, 0:128], start=False, stop=True)
        qhT_sb = consts.tile([128, NCH, 128], f32)
        nc.vector.tensor_copy(qhT_sb, ps_qh)

        # k -> khT_bf [cout%128, (chunk, j)] bf16
        k_sb = prep.tile([128, 2, HID], f32)
        nc.sync.dma_start(k_sb, k_d.rearrange("(jh p) c -> p jh c", p=128))
        k_bf = prep.tile([128, 2, HID], bf)
        nc.scalar.copy(k_bf, k_sb)
        ps_tk = psB.tile([128, NCH, NJ], bf, tag="ehT")
        for jh in range(2):
            for r in range(NCH):
                nc.tensor.transpose(ps_tk[:, r, jh * 128:(jh + 1) * 128],
                                    k_bf[:, jh, r * 128:(r + 1) * 128], ident_sb)
        kT_bf = prep.tile([128, NCH, NJ], bf)
        nc.vector.tensor_copy(kT_bf, ps_tk)
        ps_kh = psB.tile([128, NCH, NJ], f32, tag="ehT")
        for ro in range(NCH):
            for ri in range(NCH):
                nc.tensor.matmul(ps_kh[:, ro, :], w_sbs["wk"][:, ri, ro * 128:(ro + 1) * 128],
                                 kT_bf[:, ri, :], start=(ri == 0), stop=False)
            nc.tensor.matmul(ps_kh[:, ro, :], brows_sb[0:1, 1, ro * 128:(ro + 1) * 128],
                             ones_sb[0:1, :], start=False, stop=True)
        khT_bf = consts.tile([128, NCH, NJ], bf)
        nc.vector.tensor_copy(khT_bf, ps_kh)

        # v -> vh_sb [j%128, (jh, cout)] bf16 (natural layout, matvec lhsT)
        v_sb = prep.tile([128, 2, HID], f32)
        nc.sync.dma_start(v_sb, v_d.rearrange("(jh p) c -> p jh c", p=128))
        v_bf = prep.tile([128, 2, HID], bf)
        nc.scalar.copy(v_bf, v_sb)
        ps_tv = psB.tile([128, NCH, NJ], bf, tag="ehT")
        for jh in range(2):
            for r in range(NCH):
                nc.tensor.transpose(ps_tv[:, r, jh * 128:(jh + 1) * 128],
                                    v_bf[:, jh, r * 128:(r + 1) * 128], ident_sb)
        vT_bf = prep.tile([128, NCH, NJ], bf)
        nc.vector.tensor_copy(vT_bf, ps_tv)
        vh_sb = consts.tile([128, 2, HID], bf)
        for jh in range(2):
            ps_vh = psB.tile([128, HID], f32, tag="ehT")
            for ri in range(NCH):
                nc.tensor.matmul(ps_vh, vT_bf[:, ri, jh * 128:(jh + 1) * 128],
                                 w_sbs["wv"][:, ri, :], start=(ri == 0), stop=False)
            nc.tensor.matmul(ps_vh, ones_sb[0:1, 0:128], brows_sb[0:1, 2, :],
                             start=False, stop=True)
            nc.vector.tensor_copy(vh_sb[:, jh, :], ps_vh)

        # attention-output accumulator: xT [cout%128=(h%4)*32+d, (chunk=h//4, i)]
        ps_x = psX.tile([128, NCH, ILOC], f32)
        nc.vector.memset(ps_x, 0.0)

        # ---- main loop over i -----------------------------------------
        ps_w4 = None
        bias_sb = None
        for i in range(n_i):
            pos4, blk4 = i % 4, i // 4

            # load e_i fp32: [j%128, (jh, c)], then cast to bf16 on DVE
            # (SWDGE cast-DMA would need 2 sync waits; DMA pseudo-inst allows 1)
            e_f32 = p_ef.tile([128, 2, HID], f32)
            nc.sync.dma_start(e_f32, e_d[i].rearrange("(jh p) c -> p jh c", p=128))
            e_bf = p_e.tile([128, 2, HID], bf)
            nc.vector.tensor_copy(e_bf, e_f32)

            # transpose -> eT [c%128, (chunk, j)]
            ps_eT = psA.tile([128, NCH, NJ], bf, tag="eT")
            for jh in range(2):
                for r in range(NCH):
                    nc.tensor.transpose(ps_eT[:, r, jh * 128:(jh + 1) * 128],
                                        e_bf[:, jh, r * 128:(r + 1) * 128], ident_sb)
            eT_bf = p_eT.tile([128, NCH, NJ], bf)
            nc.scalar.copy(eT_bf, ps_eT)

            # mm1: ehT[cout%128, (chunk, j)] = We^T @ e^T (fp32 accum)
            ps_ehT = psB.tile([128, NCH, NJ], f32, tag="ehT")
            for ro in range(NCH):
                for ri in range(NCH):
                    nc.tensor.matmul(ps_ehT[:, ro, :], w_sbs["we"][:, ri, ro * 128:(ro + 1) * 128],
                                     eT_bf[:, ri, :], start=(ri == 0), stop=(ri == NCH - 1))

            # ehb = (ehT + be) * qh_i   (per-partition scalars, bf16 out)
            ehb_bf = p_ehb.tile([128, NCH, NJ], bf)
            for r in range(NCH):
                nc.vector.tensor_scalar(ehb_bf[:, r, :], ps_ehT[:, r, :],
                                        becol_sb[:, r:r + 1], qhT_sb[:, r, i:i + 1],
                                        op0=ALU.add, op1=ALU.mult)
            # qk_e^T = ehb * kh^T
            qk_bf = p_qk.tile([128, NCH, NJ], bf)
            nc.vector.tensor_mul(qk_bf, ehb_bf, khT_bf)

            # logits: w[h, j] for this i -> ps_w4 partitions [32*pos4, +16)
            if pos4 == 0:
                ps_w4 = psW.tile([128, NJ], f32)
                nc.vector.memset(ps_w4, 0.0)  # init head-pad partitions for softmax
                bias_sb = p_bias.tile([128, NJ], f32)
                nc.sync.dma_start(bias_sb, bias_d[blk4])
            for qq in range(NCH):
                nc.tensor.matmul(ps_w4[32 * pos4:32 * pos4 + 16, :],
                                 blk_sb[:, qq * 16:(qq + 1) * 16], qk_bf[:, qq, :],
                                 start=(qq == 0), stop=(qq == NCH - 1),
                                 tile_position=(0, 32 * pos4))

            # mm2: w_out natural [j, cout] = qk_e^T as stationary, Weo moving
            ps_wo = psB.tile([128, 2, HID], f32, tag="wout")
            for jh in range(2):
                for r in range(NCH):
                    nc.tensor.matmul(ps_wo[:, jh, :], qk_bf[:, r, jh * 128:(jh + 1) * 128],
                                     w_sbs["weo"][:, r, :], start=(r == 0), stop=(r == NCH - 1))
            wout_sb = p_wout.tile([128, 2, HID], bf)
            # single producer: the store DMA may carry only one sync wait
            nc.scalar.copy(wout_sb, ps_wo)
            nc.sync.dma_start(wout_d[i].rearrange("(jh p) c -> p jh c", p=128), wout_sb)

            # softmax + attention once per 4 i's
            if pos4 == 3:
                w_sm = p_sm.tile([128, NJ], f32, tag="w_sm")
                nc.vector.tensor_add(w_sm, ps_w4, bias_sb)
                negmax = p_small.tile([128, 1], f32, tag="negmax")
                nc.vector.tensor_reduce(negmax, w_sm, axis=AX.X, op=ALU.max, negate=True)
                a_f = p_sm.tile([128, NJ], f32, tag="a_f")
                sums = p_small.tile([128, 1], f32, tag="sums")
                nc.scalar.activation(a_f, w_sm, ACTF.Exp, bias=negmax[:, 0:1], scale=1.0,
                                     accum_out=sums[:, 0:1])
                rinv = p_small.tile([128, 1], f32, tag="rinv")
                nc.vector.reciprocal(rinv, sums)
                a_bf = p_sm.tile([128, NJ], bf, tag="a_bf")
                nc.vector.tensor_scalar_mul(a_bf, a_f, rinv[:, 0:1])

                # transpose a -> [j%128, (jh, 4i x 32)]
                ps_aT = psA.tile([128, 2, 4, 32], bf, tag="eT")
                for jh in range(2):
                    nc.tensor.transpose(ps_aT[:, jh, :, :], a_bf[:, jh * 128:(jh + 1) * 128],
                                        ident_sb)
                aT_sb = p_sm.tile([128, 2, 4, 32], bf, tag="aT")
                nc.vector.tensor_copy(aT_sb, ps_aT)

                # x[h] += a_h @ vh_h for the 4 i's of this block
                for h in range(NHEAD):
                    out_sl = ps_x[32 * (h % 4):32 * (h % 4) + 32, h // 4,
                                  4 * blk4:4 * blk4 + 4]
                    for jh in range(2):
                        nc.tensor.matmul(out_sl, vh_sb[:, jh, h * 32:(h + 1) * 32],
                                         aT_sb[:, jh, :, h], start=(jh == 0), stop=(jh == 1),
                                         tile_position=(0, 32 * (h % 4)))

        # ---- epilogue: x out-projection -------------------------------
        xT_bf = prep.tile([128, NCH, ILOC], bf)
        nc.vector.tensor_copy(xT_bf, ps_x)
        ps_xo = psB.tile([128, HID], f32, tag="ehT")
        for ri in range(NCH):
            nc.tensor.matmul(ps_xo, xT_bf[:, ri, :], w_sbs["wo"][:, ri, :],
                             start=(ri == 0), stop=False)
        nc.tensor.matmul(ps_xo, ones_sb[0:1, 0:128], brows_sb[0:1, 3, :],
                         start=False, stop=True)
        x_sb = prep.tile([128, HID], f32)
        nc.vector.tensor_copy(x_sb, ps_xo)
        nc.sync.dma_start(x_d[:, :], x_sb)

    return _patch_json_serialization(nc)


def make_host_inputs(q, k, v, e, attn_bias, Wq, bq, Wk, bk, Wv, bv, We, be,
                     Wo, bo, Weo, beo, n_i=ILOC):
    """Prepare per-core input maps (host-side layout only: slicing, weight
    chunking, bf16 cast of the small weight tensors)."""
    scale = DHEAD ** -0.5

    def chunkw(w):
        w = np.asarray(w, np.float32)
        return np.ascontiguousarray(w.reshape(NCH, 128, HID).transpose(1, 0, 2)).astype(BF)

    weights = {
        "wq": chunkw(np.asarray(Wq) * scale), "wk": chunkw(Wk), "wv": chunkw(Wv),
        "wo": chunkw(Wo), "we": chunkw(We), "weo": chunkw(Weo),
    }
    brows = np.zeros((1, 4, HID), BF)
    brows[0, 0] = (np.asarray(bq, np.float32) * scale).astype(BF)
    brows[0, 1] = np.asarray(bk, np.float32).astype(BF)
    brows[0, 2] = np.asarray(bv, np.float32).astype(BF)
    brows[0, 3] = np.asarray(bo, np.float32).astype(BF)
    becol = np.ascontiguousarray(np.asarray(be, np.float32).reshape(NCH, 128).T)
    ident = np.eye(128, dtype=BF)
    blk = np.zeros((128, NCH * NHEAD), BF)
    for qq in range(NCH):
        for cc in range(128):
            blk[cc, qq * NHEAD + 4 * qq + cc // 32] = 1.0

    nblk = n_i // 4
    in_maps = []
    for c in range(NCORES):
        b, i0 = c // 2, (c % 2) * ILOC
        ab = np.asarray(attn_bias[b, :, i0:i0 + n_i, :], np.float32)  # [16, n_i, 256]
        bias_re = np.zeros((nblk, 4, 32, NJ), np.float32)
        bias_re[:, :, :NHEAD, :] = ab.transpose(1, 0, 2).reshape(nblk, 4, NHEAD, NJ)
        m = {
            "e_loc": np.asarray(e[b, i0:i0 + n_i], np.float32),
            "q_loc": np.asarray(q[b, i0:i0 + ILOC], np.float32),
            "k_loc": np.asarray(k[b], np.float32),
            "v_loc": np.asarray(v[b], np.float32),
            "bias_loc": bias_re.reshape(nblk, 128, NJ),
            "ident": ident, "blk": blk, "brows": brows, "becol": becol,
        }
        m.update(weights)
        in_maps.append(m)
    return in_maps


def kernel(q, k, v, e, attn_bias, num_heads, Wq, bq, Wk, bk, Wv, bv, We, be,
           Wo, bo, Weo, beo):
    global _last_results
    import os

    from concourse.bass_utils import run_bass_kernel_spmd

    assert int(num_heads) == NHEAD

    in_maps = make_host_inputs(q, k, v, e, attn_bias, Wq, bq, Wk, bk, Wv, bv,
                               We, be, Wo, bo, Weo, beo)
    nc = build_nc()
    trace = os.environ.get("KERNEL_TRACE", "0") not in ("", "0")
    res = run_bass_kernel_spmd(nc, in_maps, list(range(NCORES)), trace=trace)
    _last_results = res

    x = np.empty((B, NTOK, HID), np.float32)
    wout = np.empty((B, NTOK, NJ, HID), np.float32)
    for c in range(NCORES):
        b, i0 = c // 2, (c % 2) * ILOC
        x[b, i0:i0 + ILOC] = res.results[c]["x_loc"]
        wout[b, i0:i0 + ILOC] = res.results[c]["wout_loc"].astype(np.float32)
    beo_a = np.asarray(beo, np.float32)
    if np.any(beo_a):
        wout += beo_a  # additive output-side bias, applied on host
    return (x, wout)
